# revision 1
# baseline (speedup 1.0000x reference)
"""AttentionBlock Trainium2 kernel.

Sharding: data-parallel over batch (B=8 -> one batch element per NeuronCore).
Per-core pipeline (C=512, HW=1024, 8 heads x 64):
  groupnorm (stats aggregated on-chip via indicator matmuls)
  -> QKV GEMMs (fp32r)
  -> software-pipelined per-head-pair attention: transposed scores [k, q]
     row-packed across the pair, exp on ScalarE, softmax denominators ride
     as a ones-column in the attn@v matmul
  -> normalize (reciprocal + DMA broadcast) -> proj GEMM + residual.
"""
import sys

sys.path.insert(0, "/opt/trn_rl_repo")
import numpy as np
import concourse.bass as bass
import concourse.bacc as bacc
import concourse.tile as tile
from concourse import mybir
from concourse.bass_utils import run_bass_kernel_spmd

f32 = mybir.dt.float32
f32r = mybir.dt.float32r
bf16 = mybir.dt.bfloat16
ALU = mybir.AluOpType
ACT = mybir.ActivationFunctionType

C = 512
HW = 1024
NH = 8
HD = 64
EPS = 1e-5
SCALE = HD ** -0.5
NT = C // 128  # 4 channel tiles
NP = HW // 128  # 8 position tiles


def _build():
    nc = bacc.Bacc("TRN2", target_bir_lowering=False, debug=False, num_devices=8)
    x_d = nc.dram_tensor("x", [C, HW], f32, kind="ExternalInput").ap()
    wqkvT_d = nc.dram_tensor("wqkvT", [C, 3 * C], f32, kind="ExternalInput").ap()
    wprojT_d = nc.dram_tensor("wprojT", [C, C], f32, kind="ExternalInput").ap()
    bqkv_d = nc.dram_tensor("bqkv", [3 * C], f32, kind="ExternalInput").ap()
    bproj_d = nc.dram_tensor("bproj", [C], f32, kind="ExternalInput").ap()
    gamma_d = nc.dram_tensor("gamma", [C], f32, kind="ExternalInput").ap()
    beta_d = nc.dram_tensor("beta", [C], f32, kind="ExternalInput").ap()
    gind_d = nc.dram_tensor("gind", [128, 8], f32, kind="ExternalInput").ap()
    gbc_d = nc.dram_tensor("gbc", [8, 128], f32, kind="ExternalInput").ap()
    out_d = nc.dram_tensor("out", [C, HW], f32, kind="ExternalOutput").ap()
    rs_scr = nc.dram_tensor("rs_scr", [NH, HW], f32)

    with tile.TileContext(nc) as tc:
        with (
            tc.tile_pool(name="const", bufs=1) as cp,
            tc.tile_pool(name="gnp", bufs=2) as gnp,
            tc.tile_pool(name="xp", bufs=1) as xp,
            tc.tile_pool(name="wraw", bufs=2) as wraw,
            tc.tile_pool(name="wr", bufs=1) as wr,
            tc.tile_pool(name="hp", bufs=1) as hp,
            tc.tile_pool(name="qk", bufs=1) as qkp,
            tc.tile_pool(name="vp", bufs=1) as vp,
            tc.tile_pool(name="ep", bufs=16) as ep,
            tc.tile_pool(name="ao", bufs=1) as aop,
            tc.tile_pool(name="rsb", bufs=3) as rsb,
            tc.tile_pool(name="psq", bufs=2, space="PSUM") as ps_qkv,
            tc.tile_pool(name="pss", bufs=2, space="PSUM") as ps_sc,
            tc.tile_pool(name="psa", bufs=2, space="PSUM") as ps_at,
        ):
            # ---- input loads ----
            xt = []
            for t in range(NT):
                xi = xp.tile([128, HW], f32, tag=f"x{t}")
                nc.sync.dma_start(out=xi, in_=x_d[t * 128:(t + 1) * 128, :])
                xt.append(xi)

            def col_load(src_ap, offset, name):
                t_ = cp.tile([128, NT], f32, tag=name)
                nc.gpsimd.dma_start(
                    out=t_,
                    in_=bass.AP(tensor=src_ap.tensor, offset=offset,
                                ap=[[1, 128], [128, NT]]),
                )
                return t_

            bq_sb = col_load(bqkv_d, 0, "bq")
            bk_sb = col_load(bqkv_d, C, "bk")
            gamma_sb = col_load(gamma_d, 0, "gamma")
            beta_sb = col_load(beta_d, 0, "beta")
            bproj_sb = col_load(bproj_d, 0, "bproj")
            bv_full = cp.tile([128, C], f32, tag="bv")
            nc.gpsimd.dma_start(
                out=bv_full,
                in_=bass.AP(tensor=bqkv_d.tensor, offset=2 * C,
                            ap=[[0, 128], [1, C]]),
            )
            gind_f = cp.tile([128, 8], f32, tag="gindf")
            nc.gpsimd.dma_start(out=gind_f, in_=gind_d)
            gind_r = cp.tile([128, 8], f32r, tag="gindr")
            nc.vector.tensor_copy(out=gind_r, in_=gind_f)
            gbc_f = cp.tile([8, 128], f32, tag="gbcf")
            nc.gpsimd.dma_start(out=gbc_f, in_=gbc_d)
            gbc_r = cp.tile([8, 128], f32r, tag="gbcr")
            nc.vector.tensor_copy(out=gbc_r, in_=gbc_f)

            wq_r, wp_r = [], []
            for t in range(NT):
                rwq = wraw.tile([128, 3 * C], f32, tag="wraw")
                nc.sync.dma_start(out=rwq, in_=wqkvT_d[t * 128:(t + 1) * 128, :])
                wq = wr.tile([128, 3 * C], f32r, tag=f"wq{t}")
                nc.vector.tensor_copy(out=wq, in_=rwq)
                wq_r.append(wq)
                rwp = wraw.tile([128, C], f32, tag="wpraw")
                nc.sync.dma_start(out=rwp, in_=wprojT_d[t * 128:(t + 1) * 128, :])
                wp = wr.tile([128, C], f32r, tag=f"wp{t}")
                nc.vector.tensor_copy(out=wp, in_=rwp)
                wp_r.append(wp)

            # ---- groupnorm ----
            eps_t = cp.tile([128, 1], f32, tag="eps")
            nc.vector.memset(eps_t, EPS)
            ht = []
            # per-tile 8-group sums of per-channel (mean, E) via indicator MM
            pg = ps_qkv.tile([8, 4, 2], f32, tag="qkv", name="pg")
            for t in range(NT):
                st = gnp.tile([128, 2, 6], f32, tag="bnst")
                nc.vector.bn_stats(out=st[:, 0, :], in_=xt[t][:, 0:512])
                nc.vector.bn_stats(out=st[:, 1, :], in_=xt[t][:, 512:1024])
                mv = gnp.tile([128, 2], f32, tag="mv")
                nc.vector.bn_aggr(out=mv, in_=st)
                me = gnp.tile([128, 2], f32r, tag=f"me{t}", name=f"me{t}")
                nc.vector.tensor_copy(out=me[:, 0:1], in_=mv[:, 0:1])
                sq = gnp.tile([128, 1], f32, tag="sq")
                nc.vector.tensor_mul(out=sq, in0=mv[:, 0:1], in1=mv[:, 0:1])
                nc.vector.tensor_add(out=me[:, 1:2], in0=mv[:, 1:2], in1=sq)
                nc.tensor.matmul(pg[:, t, :], lhsT=gind_r, rhs=me,
                                 start=True, stop=True)
            mE = gnp.tile([8, 4, 2], f32, tag="mE")
            nc.vector.tensor_scalar_mul(out=mE, in0=pg, scalar1=1.0 / 16.0)
            var_t = gnp.tile([8, 4], f32, tag="var")
            nc.vector.tensor_mul(out=var_t, in0=mE[:, :, 0], in1=mE[:, :, 0])
            nc.vector.tensor_sub(out=var_t, in0=mE[:, :, 1], in1=var_t)
            sd = gnp.tile([8, 4], f32, tag="sd")
            nc.scalar.activation(out=sd, in_=var_t, func=ACT.Sqrt,
                                 bias=eps_t[0:8, :], scale=1.0)
            m_rs = gnp.tile([8, 4, 2], f32r, tag="m_rs")
            nc.vector.tensor_copy(out=m_rs[:, :, 0], in_=mE[:, :, 0])
            with nc.allow_low_precision(reason="f32r rstd for matmul broadcast"):
                nc.vector.reciprocal(out=m_rs[:, :, 1], in_=sd)

            for t in range(NT):
                # broadcast (mean_g, rstd_g) to per-channel layout via matmul
                bc_ps = ps_qkv.tile([128, 2], f32, tag="qkv", name=f"bc{t}")
                nc.tensor.matmul(bc_ps, lhsT=gbc_r, rhs=m_rs[:, t, :],
                                 start=True, stop=True)
                mrt = gnp.tile([128, 2], f32, tag="mrt")
                nc.vector.tensor_copy(out=mrt, in_=bc_ps)
                A_t = gnp.tile([128, 1], f32, tag=f"A{t}")
                nc.vector.tensor_mul(out=A_t, in0=gamma_sb[:, t:t + 1],
                                     in1=mrt[:, 1:2])
                B_t = gnp.tile([128, 1], f32, tag=f"B{t}")
                tmb = gnp.tile([128, 1], f32, tag="tmb")
                nc.vector.tensor_mul(out=tmb, in0=mrt[:, 0:1], in1=A_t)
                nc.vector.tensor_sub(out=B_t, in0=beta_sb[:, t:t + 1], in1=tmb)
                # xb = x + bproj (in place); B' = B - A*bproj so h is unchanged
                nc.vector.tensor_scalar(out=xt[t], in0=xt[t],
                                        scalar1=bproj_sb[:, t:t + 1],
                                        scalar2=None, op0=ALU.add)
                tmb2 = gnp.tile([128, 1], f32, tag="tmb2")
                nc.vector.tensor_mul(out=tmb2, in0=A_t, in1=bproj_sb[:, t:t + 1])
                nc.vector.tensor_sub(out=B_t, in0=B_t, in1=tmb2)
                hh = hp.tile([128, HW], f32r, tag=f"h{t}")
                nc.vector.tensor_scalar(out=hh, in0=xt[t], scalar1=A_t,
                                        scalar2=B_t, op0=ALU.mult, op1=ALU.add)
                ht.append(hh)

            # ---- QKV GEMMs ----
            Q, K = [], []
            for m in range(NT):
                qm = qkp.tile([128, HW], f32r, tag=f"Q{m}", name=f"Q{m}")
                km = qkp.tile([128, HW], f32r, tag=f"K{m}", name=f"K{m}")
                Q.append(qm)
                K.append(km)
            for m in range(NT):
                for n in range(2):
                    ps = ps_qkv.tile([128, 512], f32, tag="qkv")
                    for kt in range(NT):
                        nc.tensor.matmul(
                            ps, lhsT=wq_r[kt][:, m * 128:(m + 1) * 128],
                            rhs=ht[kt][:, n * 512:(n + 1) * 512],
                            start=(kt == 0), stop=(kt == NT - 1))
                    nc.vector.tensor_scalar(
                        out=Q[m][:, n * 512:(n + 1) * 512], in0=ps,
                        scalar1=bq_sb[:, m:m + 1], scalar2=None, op0=ALU.add)
                for n in range(2):
                    ps = ps_qkv.tile([128, 512], f32, tag="qkv")
                    for kt in range(NT):
                        nc.tensor.matmul(
                            ps, lhsT=wq_r[kt][:, C + m * 128:C + (m + 1) * 128],
                            rhs=ht[kt][:, n * 512:(n + 1) * 512],
                            start=(kt == 0), stop=(kt == NT - 1))
                    nc.vector.tensor_scalar(
                        out=K[m][:, n * 512:(n + 1) * 512], in0=ps,
                        scalar1=bk_sb[:, m:m + 1], scalar2=None, op0=ALU.add)

            vT = []
            for p8 in range(NP):
                vtile = vp.tile([128, NH, HD + 1], bf16, tag=f"vT{p8}",
                                name=f"vT{p8}")
                nc.vector.memset(vtile[:, :, HD:HD + 1], 1.0)
                ps = ps_qkv.tile([128, 512], f32, tag="qkv")
                for kt in range(NT):
                    nc.tensor.matmul(
                        ps, lhsT=ht[kt][:, p8 * 128:(p8 + 1) * 128],
                        rhs=wq_r[kt][:, 2 * C:3 * C],
                        start=(kt == 0), stop=(kt == NT - 1))
                nc.vector.tensor_add(
                    out=vtile[:, :, 0:HD],
                    in0=ps.rearrange("p (h d) -> p h d", h=NH),
                    in1=bv_full.rearrange("p (h d) -> p h d", h=NH))
                vT.append(vtile)

            # ---- attention, software-pipelined head pairs ----
            attO = [aop.tile([128, HW], f32, tag=f"ao{ct}", name=f"ao{ct}")
                    for ct in range(NT)]
            # head h's denominators at partition 32*(h//2), column block h%2
            sums_all = cp.tile([128, 2 * HW], f32, tag="sums")

            def emit_scores(pr):
                ct = pr
                epair = {0: [], 1: []}
                for kk in range(NP):
                    pss = {}
                    for hb in range(2):
                        off = 64 * hb
                        ps = ps_sc.tile([128, 1024], f32, tag="sc",
                                        name=f"sc{pr}_{kk}_{hb}")
                        for n in range(2):
                            nc.tensor.matmul(
                                ps[:, n * 512:(n + 1) * 512],
                                lhsT=K[ct][off:off + 64, kk * 128:(kk + 1) * 128],
                                rhs=Q[ct][off:off + 64, n * 512:(n + 1) * 512],
                                start=True, stop=True)
                        pss[hb] = ps
                    for hb in range(2):
                        et = ep.tile([128, 1024], bf16, tag="expT",
                                     name=f"et{pr}_{kk}_{hb}")
                        nc.scalar.activation(out=et, in_=pss[hb], func=ACT.Exp,
                                             scale=SCALE)
                        epair[hb].append(et)
                return epair

            def emit_av(pr, epair):
                ct = pr
                for hb in range(2):
                    h_ = 2 * pr + hb
                    off = 64 * hb
                    sp = 32 * (h_ // 2)
                    sb_ = (h_ % 2) * HW
                    for n in range(2):
                        pa = ps_at.tile([128, 512], f32, tag="at",
                                        name=f"at{pr}_{hb}_{n}")
                        for kk in range(NP):
                            nc.tensor.matmul(
                                pa[0:HD + 1, :],
                                lhsT=vT[kk][:, h_, :],
                                rhs=epair[hb][kk][:, n * 512:(n + 1) * 512],
                                start=(kk == 0), stop=(kk == NP - 1))
                        nc.vector.tensor_copy(
                            out=attO[ct].bitcast(f32r)[off:off + 64,
                                                       n * 512:(n + 1) * 512],
                            in_=pa[0:HD, :])
                        nc.vector.tensor_copy(
                            out=sums_all[sp:sp + 1,
                                         sb_ + n * 512:sb_ + (n + 1) * 512],
                            in_=pa[HD:HD + 1, :])

            prev = emit_scores(0)
            for pr in range(NH // 2):
                nxt = emit_scores(pr + 1) if pr + 1 < NH // 2 else None
                emit_av(pr, prev)
                prev = nxt

            # ---- normalize + proj + residual ----
            rs_all = cp.tile([128, 2 * HW], f32, tag="rs")
            rs_view = rs_all.rearrange("(a b) (c q) -> a b c q",
                                       b=32, q=HW)[:, 0, :, :]
            for half in range(2):
                nc.vector.reciprocal(out=rs_all[64 * half:64 * half + 64, :],
                                     in_=sums_all[64 * half:64 * half + 64, :])
                nc.sync.dma_start(out=rs_scr[4 * half:4 * half + 4, :],
                                  in_=rs_view[2 * half:2 * half + 2, :, :])
            for ct in range(NT):
                rb = rsb.tile([128, HW], f32, tag="rsb")
                nc.gpsimd.dma_start(
                    out=rb,
                    in_=bass.AP(tensor=rs_scr, offset=ct * 2 * HW,
                                ap=[[HW, 2], [0, 64], [1, HW]]),
                )
                nc.vector.tensor_mul(out=attO[ct].bitcast(f32r),
                                     in0=attO[ct], in1=rb)
            for m in range(NT):
                for n in range(2):
                    ps = ps_qkv.tile([128, 512], f32, tag="qkv")
                    for ct in range(NT):
                        nc.tensor.matmul(
                            ps, lhsT=wp_r[ct][:, m * 128:(m + 1) * 128],
                            rhs=attO[ct].bitcast(f32r)[:, n * 512:(n + 1) * 512],
                            start=(ct == 0), stop=(ct == NT - 1))
                    nc.vector.tensor_add(
                        out=xt[m][:, n * 512:(n + 1) * 512], in0=ps,
                        in1=xt[m][:, n * 512:(n + 1) * 512])
                    nc.sync.dma_start(
                        out=out_d[m * 128:(m + 1) * 128,
                                  n * 512:(n + 1) * 512],
                        in_=xt[m][:, n * 512:(n + 1) * 512])
    nc.compile()
    return nc


_NC = None


def kernel(x, gamma, beta, w_qkv, b_qkv, w_proj, b_proj):
    global _NC
    x = np.asarray(x, dtype=np.float32)
    B = x.shape[0]
    assert B == 8
    if _NC is None:
        _NC = _build()
    wqkvT = np.ascontiguousarray(np.asarray(w_qkv, np.float32).T)
    wprojT = np.ascontiguousarray(np.asarray(w_proj, np.float32).T)
    common = {
        "gind": np.ascontiguousarray(
            np.repeat(np.eye(8, dtype=np.float32), 16, axis=0)),
        "gbc": np.ascontiguousarray(
            np.repeat(np.eye(8, dtype=np.float32), 16, axis=1)),
        "wqkvT": wqkvT,
        "wprojT": wprojT,
        "bqkv": np.asarray(b_qkv, np.float32),
        "bproj": np.asarray(b_proj, np.float32),
        "gamma": np.asarray(gamma, np.float32),
        "beta": np.asarray(beta, np.float32),
    }
    in_maps = [
        {"x": np.ascontiguousarray(x[b].reshape(C, HW)), **common}
        for b in range(B)
    ]
    res = run_bass_kernel_spmd(_NC, in_maps, core_ids=list(range(8)))
    out = np.stack([res.results[b]["out"] for b in range(B)])
    return out.reshape(B, C, 32, 32).astype(np.float32)

